# revision 29
# baseline (speedup 1.0000x reference)
"""Causal GQA self-attention (B=2, T=2048, C=1024, 16 q-heads / 4 kv-heads,
RoPE, causal softmax, output projection) on 8 Trainium2 NeuronCores.

Sharding: core c = b*4 + hg handles batch b (2-way data parallel) and
head-group hg (4-way tensor parallel: its 4 q-heads + their shared kv head).
W_qkv is column-sharded, W_proj row-sharded; each core emits a partial
projection [2048, 1024] and the host sums the 4 partials per batch.

Device pipeline per core (bf16 matmul inputs, fp32 PSUM accumulate):
  1. qkv = x @ W_qkv_shard -> token-major tiles q01 [128,4t,256] (4 q heads)
     and kvv [128,4t,256] = k | k-dup | v | ones | pad
  2. RoPE in token-major (head-dim pairs pre-permuted even|odd on the host);
     k roped once then duplicated
  3. TWO multi-tile DMA-xbar transposes per 512-token chunk (block-fold 3D
     output form): q01 -> qt (all 4 heads in one instruction), kvv -> ktT
  4. flash-style per 512-wide q chunk: S^T = k @ qT for both heads of a pair
     into one [128,2,512] PSUM tile; ONE exp over the pair, routed between
     ScalarE (exact exp) and DVE (1-op Schraudolph bf16-exp2: u16 =
     a*s + b convert, bitcast to bf16) to balance engines; causal masking
     only on diagonal 128-blocks; y^T[64+1, q] += [v|1]^T @ P^T
  5. y^T row 64 is the softmax denominator: reciprocal_approx straight off
     PSUM + GPSIMD partition_broadcast + fused normalize-evacuate multiply
  6. out = yT.T @ W_proj_shard  (paired [128,2,512] PSUM, single evacuate)
"""

import sys

if "/opt/trn_rl_repo" not in sys.path:
    sys.path.insert(0, "/opt/trn_rl_repo")

import numpy as np
import ml_dtypes

B, T, C = 2, 2048, 1024
NH, NKV, HD = 16, 4, 64
THETA = 10000.0
NQ = NH // NKV          # q heads per core = 4
TT = T // 128           # 16 token tiles
NCHUNK = T // 512       # 4 q-chunks
BF16 = ml_dtypes.bfloat16

# Schraudolph bf16 exp2: u16 = trunc(A*s_raw + B); bitcast -> bf16
# approximates exp(s_raw/8).  A = 128*log2(e)/8; B tuned for min rms rel err
# under truncation (see tuning sweep: shift -7 -> rms 1.78%, mean ~0).
EXP_A = 128.0 * 1.4426950408889634 * 0.125
EXP_B = 127.0 * 128.0 - 7.0
EXP_DVE_FRAC = 0.0     # fraction of exp columns routed to DVE

_CACHE = {}


def _build():
    """Build the SPMD Bass program (identical on all 8 cores)."""
    import concourse.mybir as mybir
    import concourse.tile as tile
    from concourse import bacc
    from concourse.bass import ts
    from contextlib import ExitStack

    dt = mybir.dt
    AF = mybir.ActivationFunctionType

    nc = bacc.Bacc("TRN2", target_bir_lowering=False, debug=False, num_devices=8)

    xt_d = nc.declare_dram_parameter("xT", [128, 8 * T], dt.bfloat16, isOutput=False)
    w_d = nc.declare_dram_parameter("w384", [128, 8 * 384], dt.bfloat16, isOutput=False)
    wo_d = nc.declare_dram_parameter("wo", [128, 2 * C], dt.bfloat16, isOutput=False)
    cs_d = nc.declare_dram_parameter(
        "cs", [128, NCHUNK * 4 * 128], dt.bfloat16, isOutput=False
    )
    out_d = nc.declare_dram_parameter("out", [T, C], dt.bfloat16, isOutput=True)

    with tile.TileContext(nc) as tc, ExitStack() as ctx:
        persist = ctx.enter_context(tc.tile_pool(name="persist", bufs=1))
        rope_tmp = ctx.enter_context(tc.tile_pool(name="rope_tmp", bufs=2))
        p_pool = ctx.enter_context(tc.tile_pool(name="p_pool", bufs=8))
        po_pool = ctx.enter_context(tc.tile_pool(name="po", bufs=4))
        bc_pool = ctx.enter_context(tc.tile_pool(name="bc", bufs=2))
        den_pool = ctx.enter_context(tc.tile_pool(name="den", bufs=2))
        yst_pool = ctx.enter_context(tc.tile_pool(name="yst", bufs=6))
        s_ps_pool = ctx.enter_context(tc.tile_pool(name="s_ps", bufs=4, space="PSUM"))
        y_ps_pool = ctx.enter_context(tc.tile_pool(name="y_ps", bufs=1, space="PSUM"))

        # ---- persistent SBUF; loads are fine-grained + just-in-time so the
        # first qkv matmuls and the chunk-0 transposes aren't queued behind
        # bulk transfers the kernel won't need until later ----
        w_sb = persist.tile([128, 8, 384], dt.bfloat16)
        w_rearr = w_d.ap().rearrange("p (c n) -> p c n", c=8)
        nc.sync.dma_start(w_sb[:, 0:1, :], w_rearr[:, 0:1, :])
        xt_sb = []
        cs_sb = []
        for jc in range(NCHUNK):
            xt_sb.append(persist.tile([128, 8, 512], dt.bfloat16, name=f"xtc{jc}"))
            cs_sb.append(persist.tile([128, 4, 128], dt.bfloat16, name=f"cs{jc}"))
        xt0_re = xt_d.ap()[:, ts(0, 8 * 512)].rearrange("p (c t) -> p c t", c=8)
        nc.sync.dma_start(xt_sb[0][:, 0:2, :], xt0_re[:, 0:2, :])
        nc.sync.dma_start(w_sb[:, 1:4, :], w_rearr[:, 1:4, :])
        nc.sync.dma_start(xt_sb[0][:, 2:4, :], xt0_re[:, 2:4, :])
        nc.sync.dma_start(w_sb[:, 4:8, :], w_rearr[:, 4:8, :])
        nc.sync.dma_start(xt_sb[0][:, 4:8, :], xt0_re[:, 4:8, :])
        nc.sync.dma_start(
            cs_sb[0][:],
            cs_d.ap()[:, ts(0, 4 * 128)].rearrange("p (n d) -> p n d", n=4),
        )
        def emit_loads(jc):
            xt_re = xt_d.ap()[:, ts(jc, 8 * 512)].rearrange("p (c t) -> p c t", c=8)
            nc.sync.dma_start(xt_sb[jc][:, 0:4, :], xt_re[:, 0:4, :])
            nc.sync.dma_start(xt_sb[jc][:, 4:8, :], xt_re[:, 4:8, :])
            nc.sync.dma_start(
                cs_sb[jc][:],
                cs_d.ap()[:, ts(jc, 4 * 128)].rearrange("p (n d) -> p n d", n=4),
            )

        emit_loads(1)
        wo_sb = persist.tile([128, 2, C], dt.bfloat16)

        kvv_sb = []  # per chunk [128,4,256]: k | kdup | v | ones | pad
        qt_sb = []   # per chunk [128, 4t, 2dh, 128tok]; head = 2*dh + part//64
        kt_sb = []   # per chunk [128, 4t, 2half, 128]: half0 rows = kT|kT-dup
        ynt = [[None] * NCHUNK for _ in range(2)]
        for d in range(2):
            for j in range(NCHUNK):
                ynt[d][j] = persist.tile([128, 512], dt.bfloat16, name=f"ynt{d}_{j}")

        # exp routing state: greedy column balance toward EXP_DVE_FRAC
        route = {"tot": 0.0, "dve": 0.0}

        def emit_phase1(jc):
            if jc >= 2:
                emit_loads(jc)
            if jc == 2:
                nc.sync.dma_start(
                    wo_sb[:], wo_d.ap().rearrange("p (c n) -> p c n", c=2)
                )
            q01 = persist.tile([128, 4, 256], dt.bfloat16, name=f"q01_{jc}")
            kvv = persist.tile([128, 4, 256], dt.bfloat16, name=f"kvv{jc}")
            kvv_sb.append(kvv)
            for t4 in range(4):
                ps = y_ps_pool.tile(
                    [128, 384], dt.float32, tag=f"y{t4 % 2}", name="qkv_ps"
                )
                for c in range(8):
                    nc.tensor.matmul(
                        ps[:],
                        lhsT=xt_sb[jc][:, c, ts(t4, 128)],
                        rhs=w_sb[:, c, :],
                        start=(c == 0),
                        stop=(c == 7),
                    )
                nc.vector.tensor_copy(q01[:, t4, :], ps[:, 0:256])
                # k -> cols 0:64, v -> cols 128:192 in one strided copy
                dst = kvv[:, t4, 0:256].rearrange("p (b two d) -> p b two d", b=2, two=2)
                nc.scalar.copy(dst[:, :, 0, :], ps[:, 256:384].rearrange("p (b d) -> p b d", b=2))
            nc.gpsimd.memset(kvv[:, :, 192:256], 1.0)  # ones col + pad

            # RoPE in place; tables tiled x2 heads on host, loop head-pairs
            csc = cs_sb[jc][:]
            qcos = csc[:, :, 0:64].rearrange("p f (h d) -> p f h d", h=2)
            qsin = csc[:, :, 64:128].rearrange("p f (h d) -> p f h d", h=2)
            qv = q01[:].rearrange("p f (h d) -> p f h d", h=4)
            t1 = rope_tmp.tile([128, 4, 4, 32], dt.bfloat16, tag="t1")
            t2 = rope_tmp.tile([128, 4, 4, 32], dt.bfloat16, tag="t2")
            t3 = rope_tmp.tile([128, 4, 4, 32], dt.bfloat16, tag="t3")
            t4_ = rope_tmp.tile([128, 4, 4, 32], dt.bfloat16, tag="t4")
            for hh in range(2):
                x1 = qv[:, :, ts(hh, 2), 0:32]
                x2 = qv[:, :, ts(hh, 2), 32:64]
                a1 = t1[:, :, ts(hh, 2), :]
                a2 = t2[:, :, ts(hh, 2), :]
                a3 = t3[:, :, ts(hh, 2), :]
                a4 = t4_[:, :, ts(hh, 2), :]
                nc.vector.tensor_mul(a1, x1, qcos)
                nc.vector.tensor_mul(a2, x2, qsin)
                nc.vector.tensor_mul(a3, x1, qsin)
                nc.vector.tensor_mul(a4, x2, qcos)
                nc.vector.tensor_sub(x1, a1, a2)
                nc.vector.tensor_add(x2, a3, a4)
            kx1 = kvv[:, :, 0:32]
            kx2 = kvv[:, :, 32:64]
            kcos = csc[:, :, 0:32]
            ksin = csc[:, :, 64:96]
            k1 = rope_tmp.tile([128, 4, 32], dt.bfloat16, tag="k1")
            k2 = rope_tmp.tile([128, 4, 32], dt.bfloat16, tag="k2")
            k3 = rope_tmp.tile([128, 4, 32], dt.bfloat16, tag="k3")
            k4 = rope_tmp.tile([128, 4, 32], dt.bfloat16, tag="k4")
            nc.vector.tensor_mul(k1[:], kx1, kcos)
            nc.vector.tensor_mul(k2[:], kx2, ksin)
            nc.vector.tensor_mul(k3[:], kx1, ksin)
            nc.vector.tensor_mul(k4[:], kx2, kcos)
            nc.vector.tensor_sub(kx1, k1[:], k2[:])
            nc.vector.tensor_add(kx2, k3[:], k4[:])
            nc.scalar.copy(kvv[:, :, 64:128], kvv[:, :, 0:64])

            qt = persist.tile([128, 4, 2, 128], dt.bfloat16, name=f"qt{jc}")
            nc.sync.dma_start_transpose(qt[:], q01[:].rearrange("p a b -> p (a b)"))
            ktT = persist.tile([128, 4, 2, 128], dt.bfloat16, name=f"ktT{jc}")
            nc.sync.dma_start_transpose(ktT[:], kvv[:].rearrange("p a b -> p (a b)"))
            qt_sb.append(qt)
            kt_sb.append(ktT)

        def emit_attention(j, hps=(0, 1), defer_last_norm=False):
            deferred = None
            for hp in hps:
                y_ps = y_ps_pool.tile(
                    [65, 2, 512], dt.float32, tag=f"y{hp}", name=f"y_ps{hp}"
                )
                last = 4 * j + 3
                pending = []  # deferred y matmuls: (i, ic, i4, off, p_t)

                def flush_y():
                    i, ic, i4, off, p_parts = pending.pop(0)
                    for u in range(2):
                        nc.tensor.matmul(
                            y_ps[:, u, off:512],
                            lhsT=kvv_sb[ic][:, i4, 128:193],
                            rhs=p_parts[u][:, off:512],
                            start=(i == 0),
                            stop=(i == last),
                        )

                for i in range(4 * j + 4):  # k tiles
                    ic, i4 = divmod(i, 4)
                    off = max(0, 128 * i - 512 * j)
                    w = 512 - off
                    p_parts = []
                    for u in range(2):  # head 2hp+u; kT copy at partitions 64u
                        s_ps = s_ps_pool.tile(
                            [128, 512], dt.float32, tag="s", name="s_ps"
                        )
                        nc.tensor.matmul(
                            s_ps[:, off:512],
                            lhsT=kt_sb[ic][ts(u, 64), i4, 0, :],
                            rhs=qt_sb[j][ts(u, 64), off // 128 : 4, hp, :],
                            start=True,
                            stop=True,
                        )
                        # route exp: ScalarE exact vs DVE Schraudolph
                        route["tot"] += w
                        use_dve = route["dve"] < EXP_DVE_FRAC * route["tot"]
                        if use_dve:
                            route["dve"] += w
                            p_u = p_pool.tile([128, 512], dt.uint16, name="p_u")
                            nc.vector.tensor_scalar(
                                p_u[:, off:512],
                                s_ps[:, off:512],
                                EXP_A,
                                EXP_B,
                                mybir.AluOpType.mult,
                                mybir.AluOpType.add,
                            )
                            p_t = p_u[:].bitcast(dt.bfloat16)
                        else:
                            p_b = p_pool.tile([128, 512], dt.bfloat16, name="p_b")
                            nc.scalar.activation(
                                p_b[:, off:512], s_ps[:, off:512], AF.Exp, scale=0.125
                            )
                            p_t = p_b[:]
                        if 128 * i >= 512 * j:  # diagonal block: causal mask
                            nc.gpsimd.affine_select(
                                p_t[:, off : off + 128],
                                p_t[:, off : off + 128],
                                pattern=[[1, 128]],
                                compare_op=mybir.AluOpType.is_ge,
                                fill=0.0,
                                base=0,
                                channel_multiplier=-1,
                            )
                        p_parts.append(p_t)
                    pending.append((i, ic, i4, off, p_parts))
                    if len(pending) > 1:
                        flush_y()
                while pending:
                    flush_y()
                # y rows 0:64 = v dims, row 64 = denominator.  The muls
                # write ynt's upper/lower partition halves DIRECTLY (DVE APs
                # carry independent partition bases) - no staging DMA.
                def emit_norm(hp=hp, y_ps=y_ps):
                    den = den_pool.tile([1, 2, 512], dt.float32)
                    nc.vector.reciprocal_approx_fast(den[:], y_ps[64:65, :, :])
                    bc = bc_pool.tile([64, 2, 512], dt.float32)
                    nc.gpsimd.partition_broadcast(bc[:], den[:], channels=64)
                    for u in range(2):
                        yst = yst_pool.tile([64, 512], dt.bfloat16)
                        nc.vector.tensor_mul(yst[:, :], y_ps[0:64, u, :], bc[:, u, :])
                        nc.scalar.dma_start(ynt[hp][j][ts(u, 64), :], yst[:])
                if defer_last_norm and hp == hps[-1]:
                    deferred = emit_norm
                else:
                    emit_norm()
            return deferred

        def emit_proj(j):
            for t4 in range(4):
                tt = 4 * j + t4
                po = po_pool.tile([128, 1024], dt.bfloat16)
                for nn2 in range(2):
                    ps = s_ps_pool.tile([128, 512], dt.float32, tag="s", name="pr_ps")
                    for dtile in range(2):
                        nc.tensor.matmul(
                            ps[:],
                            lhsT=ynt[dtile][j][:, ts(t4, 128)],
                            rhs=wo_sb[:, dtile, ts(nn2, 512)],
                            start=(dtile == 0),
                            stop=(dtile == 1),
                        )
                    if nn2 == 0:
                        nc.scalar.copy(po[:, ts(nn2, 512)], ps[:])
                    else:
                        nc.vector.tensor_copy(po[:, ts(nn2, 512)], ps[:])
                nc.scalar.dma_start(out_d.ap()[ts(tt, 128), :], po[:])

        # ---- interleaved emission: each engine-queue boundary is covered by
        # independent PE work (qkv of a later chunk, proj of an earlier one);
        # attention(j) is emitted before phase1(j+1) so its score matmuls
        # never wait on later chunks' transposes (per-queue coalesced deps) --
        emit_phase1(0)
        emit_phase1(1)
        emit_attention(0)
        n1 = emit_attention(1, defer_last_norm=True)
        emit_phase1(2)
        emit_proj(0)
        n1()
        emit_phase1(3)
        n2 = emit_attention(2, defer_last_norm=True)
        emit_proj(1)
        n2()
        n30 = emit_attention(3, hps=(0,), defer_last_norm=True)
        emit_proj(2)
        n30()
        emit_attention(3, hps=(1,))
        emit_proj(3)

    nc.finalize()
    return nc


def _host_inputs(x, W_qkv, W_proj):
    """Per-core input maps (host-side sharding + partition-major layout)."""
    perm = np.concatenate([np.arange(0, HD, 2), np.arange(1, HD, 2)])  # even|odd
    inv = 1.0 / THETA ** (np.arange(0, HD, 2, dtype=np.float64) / HD)  # [32]
    ang = np.arange(T, dtype=np.float64)[:, None] * inv[None, :]       # [T, 32]
    cos2 = np.tile(np.cos(ang), (1, 2))
    sin2 = np.tile(np.sin(ang), (1, 2))
    cs = np.concatenate([cos2, sin2], axis=1).astype(BF16)             # [T, 128]
    cs_pm = np.ascontiguousarray(
        cs.reshape(TT, 128, 128).transpose(1, 0, 2).reshape(128, TT * 128)
    )

    def part_major(a, p=128):
        R, cols = a.shape
        n = R // p
        return np.ascontiguousarray(
            a.reshape(n, p, cols).transpose(1, 0, 2).reshape(p, n * cols)
        )

    in_maps = []
    for core in range(8):
        b, hg = divmod(core, 4)
        xT = x[b].T.astype(BF16)                                       # [C, T]
        xt_pm = np.ascontiguousarray(
            xT.reshape(8, 128, NCHUNK, 512)
            .transpose(1, 2, 0, 3)
            .reshape(128, NCHUNK * 8 * 512)
        )
        cols = []
        for h in range(hg * NQ, hg * NQ + NQ):
            cols.append(W_qkv[:, h * HD : (h + 1) * HD][:, perm])
        kblk = W_qkv[:, NH * HD + hg * HD : NH * HD + (hg + 1) * HD][:, perm]
        vblk = W_qkv[:, (NH + NKV) * HD + hg * HD : (NH + NKV) * HD + (hg + 1) * HD]
        w384 = np.concatenate(cols + [kblk, vblk], axis=1).astype(BF16)
        wo = W_proj[hg * NQ * HD : (hg + 1) * NQ * HD, :].astype(BF16)
        in_maps.append(
            {
                "xT": xt_pm,
                "w384": part_major(w384),
                "wo": part_major(wo),
                "cs": cs_pm,
            }
        )
    return in_maps


def _run(in_maps):
    from concourse.bass_utils import run_bass_kernel_spmd

    if "nc" not in _CACHE:
        _CACHE["nc"] = _build()
    return run_bass_kernel_spmd(_CACHE["nc"], in_maps, core_ids=list(range(8)))


def kernel(x, W_qkv, W_proj):
    x = np.asarray(x, dtype=np.float32)
    W_qkv = np.asarray(W_qkv, dtype=np.float32)
    W_proj = np.asarray(W_proj, dtype=np.float32)
    res = _run(_host_inputs(x, W_qkv, W_proj))
    out = np.zeros((B, T, C), dtype=np.float32)
    for core in range(8):
        b = core // 4
        out[b] += res.results[core]["out"].astype(np.float32)
    return out


# revision 31
# speedup vs baseline: 2.8072x; 2.8072x over previous
"""Causal GQA self-attention (B=2, T=2048, C=1024, 16 q-heads / 4 kv-heads,
RoPE, causal softmax, output projection) on 8 Trainium2 NeuronCores.

Sharding: core c = b*4 + hg handles batch b (2-way data parallel) and
head-group hg (4-way tensor parallel: its 4 q-heads + their shared kv head).
W_qkv is column-sharded, W_proj row-sharded; each core emits a partial
projection [2048, 1024] and the host sums the 4 partials per batch.

Device pipeline per core (bf16 matmul inputs, fp32 PSUM accumulate):
  1. qkv = x @ W_qkv_shard -> token-major tiles q01 [128,4t,256] (4 q heads)
     and kvv [128,4t,256] = k | k-dup | v | ones | pad
  2. RoPE in token-major (head-dim pairs pre-permuted even|odd on the host);
     k roped once then duplicated
  3. TWO multi-tile DMA-xbar transposes per 512-token chunk (block-fold 3D
     output form): q01 -> qt (all 4 heads in one instruction), kvv -> ktT
  4. flash-style per 512-wide q chunk: S^T = k @ qT for both heads of a pair
     into one [128,2,512] PSUM tile; ONE exp over the pair, routed between
     ScalarE (exact exp) and DVE (1-op Schraudolph bf16-exp2: u16 =
     a*s + b convert, bitcast to bf16) to balance engines; causal masking
     only on diagonal 128-blocks; y^T[64+1, q] += [v|1]^T @ P^T
  5. y^T row 64 is the softmax denominator: reciprocal_approx straight off
     PSUM + GPSIMD partition_broadcast + fused normalize-evacuate multiply
  6. out = yT.T @ W_proj_shard  (paired [128,2,512] PSUM, single evacuate)
"""

import sys

if "/opt/trn_rl_repo" not in sys.path:
    sys.path.insert(0, "/opt/trn_rl_repo")

import numpy as np
import ml_dtypes

B, T, C = 2, 2048, 1024
NH, NKV, HD = 16, 4, 64
THETA = 10000.0
NQ = NH // NKV          # q heads per core = 4
TT = T // 128           # 16 token tiles
NCHUNK = T // 512       # 4 q-chunks
BF16 = ml_dtypes.bfloat16

# Schraudolph bf16 exp2: u16 = trunc(A*s_raw + B); bitcast -> bf16
# approximates exp(s_raw/8).  A = 128*log2(e)/8; B tuned for min rms rel err
# under truncation (see tuning sweep: shift -7 -> rms 1.78%, mean ~0).
EXP_A = 128.0 * 1.4426950408889634 * 0.125
EXP_B = 127.0 * 128.0 - 7.0
EXP_DVE_FRAC = 0.0     # fraction of exp columns routed to DVE

_CACHE = {}


def _build():
    """Build the SPMD Bass program (identical on all 8 cores)."""
    import concourse.mybir as mybir
    import concourse.tile as tile
    from concourse import bacc
    from concourse.bass import ts
    from contextlib import ExitStack

    dt = mybir.dt
    AF = mybir.ActivationFunctionType

    nc = bacc.Bacc("TRN2", target_bir_lowering=False, debug=False, num_devices=8)

    xt_d = nc.declare_dram_parameter("xT", [128, 8 * T], dt.bfloat16, isOutput=False)
    w_d = nc.declare_dram_parameter("w384", [128, 8 * 384], dt.bfloat16, isOutput=False)
    wo_d = nc.declare_dram_parameter("wo", [128, 2 * C], dt.bfloat16, isOutput=False)
    cs_d = nc.declare_dram_parameter(
        "cs", [128, NCHUNK * 4 * 128], dt.bfloat16, isOutput=False
    )
    out_d = nc.declare_dram_parameter("out", [T, C], dt.bfloat16, isOutput=True)

    with tile.TileContext(nc) as tc, ExitStack() as ctx:
        persist = ctx.enter_context(tc.tile_pool(name="persist", bufs=1))
        rope_tmp = ctx.enter_context(tc.tile_pool(name="rope_tmp", bufs=2))
        p_pool = ctx.enter_context(tc.tile_pool(name="p_pool", bufs=8))
        po_pool = ctx.enter_context(tc.tile_pool(name="po", bufs=4))
        bc_pool = ctx.enter_context(tc.tile_pool(name="bc", bufs=2))
        den_pool = ctx.enter_context(tc.tile_pool(name="den", bufs=2))
        yst_pool = ctx.enter_context(tc.tile_pool(name="yst", bufs=6))
        s_ps_pool = ctx.enter_context(tc.tile_pool(name="s_ps", bufs=4, space="PSUM"))
        y_ps_pool = ctx.enter_context(tc.tile_pool(name="y_ps", bufs=1, space="PSUM"))

        # ---- persistent SBUF; loads are fine-grained + just-in-time so the
        # first qkv matmuls and the chunk-0 transposes aren't queued behind
        # bulk transfers the kernel won't need until later ----
        w_sb = persist.tile([128, 8, 384], dt.bfloat16)
        nc.sync.dma_start(w_sb[:], w_d.ap().rearrange("p (c n) -> p c n", c=8))
        xt_sb = []
        cs_sb = []
        for jc in range(NCHUNK):
            xt = persist.tile([128, 8, 512], dt.bfloat16, name=f"xtc{jc}")
            nc.sync.dma_start(
                xt[:],
                xt_d.ap()[:, ts(jc, 8 * 512)].rearrange("p (c t) -> p c t", c=8),
            )
            xt_sb.append(xt)
            cst = persist.tile([128, 4, 128], dt.bfloat16, name=f"cs{jc}")
            nc.sync.dma_start(
                cst[:],
                cs_d.ap()[:, ts(jc, 4 * 128)].rearrange("p (n d) -> p n d", n=4),
            )
            cs_sb.append(cst)
        wo_sb = persist.tile([128, 2, C], dt.bfloat16)
        nc.sync.dma_start(wo_sb[:], wo_d.ap().rearrange("p (c n) -> p c n", c=2))

        kvv_sb = []  # per chunk [128,4,256]: k | kdup | v | ones | pad
        qt_sb = []   # per chunk [128, 4t, 2dh, 128tok]; head = 2*dh + part//64
        kt_sb = []   # per chunk [128, 4t, 2half, 128]: half0 rows = kT|kT-dup
        ynt = [[None] * NCHUNK for _ in range(2)]
        for d in range(2):
            for j in range(NCHUNK):
                ynt[d][j] = persist.tile([128, 512], dt.bfloat16, name=f"ynt{d}_{j}")

        # exp routing state: greedy column balance toward EXP_DVE_FRAC
        route = {"tot": 0.0, "dve": 0.0}

        def emit_phase1(jc):
            q01 = persist.tile([128, 4, 256], dt.bfloat16, name=f"q01_{jc}")
            kvv = persist.tile([128, 4, 256], dt.bfloat16, name=f"kvv{jc}")
            kvv_sb.append(kvv)
            for t4 in range(4):
                ps = y_ps_pool.tile(
                    [128, 384], dt.float32, tag=f"y{t4 % 2}", name="qkv_ps"
                )
                for c in range(8):
                    nc.tensor.matmul(
                        ps[:],
                        lhsT=xt_sb[jc][:, c, ts(t4, 128)],
                        rhs=w_sb[:, c, :],
                        start=(c == 0),
                        stop=(c == 7),
                    )
                nc.vector.tensor_copy(q01[:, t4, :], ps[:, 0:256])
                # k -> cols 0:64, v -> cols 128:192 in one strided copy
                dst = kvv[:, t4, 0:256].rearrange("p (b two d) -> p b two d", b=2, two=2)
                nc.scalar.copy(dst[:, :, 0, :], ps[:, 256:384].rearrange("p (b d) -> p b d", b=2))
            nc.gpsimd.memset(kvv[:, :, 192:256], 1.0)  # ones col + pad

            # RoPE in place; tables tiled x2 heads on host, loop head-pairs
            csc = cs_sb[jc][:]
            qcos = csc[:, :, 0:64].rearrange("p f (h d) -> p f h d", h=2)
            qsin = csc[:, :, 64:128].rearrange("p f (h d) -> p f h d", h=2)
            qv = q01[:].rearrange("p f (h d) -> p f h d", h=4)
            t1 = rope_tmp.tile([128, 4, 4, 32], dt.bfloat16, tag="t1")
            t2 = rope_tmp.tile([128, 4, 4, 32], dt.bfloat16, tag="t2")
            t3 = rope_tmp.tile([128, 4, 4, 32], dt.bfloat16, tag="t3")
            t4_ = rope_tmp.tile([128, 4, 4, 32], dt.bfloat16, tag="t4")
            for hh in range(2):
                x1 = qv[:, :, ts(hh, 2), 0:32]
                x2 = qv[:, :, ts(hh, 2), 32:64]
                a1 = t1[:, :, ts(hh, 2), :]
                a2 = t2[:, :, ts(hh, 2), :]
                a3 = t3[:, :, ts(hh, 2), :]
                a4 = t4_[:, :, ts(hh, 2), :]
                nc.vector.tensor_mul(a1, x1, qcos)
                nc.vector.tensor_mul(a2, x2, qsin)
                nc.vector.tensor_mul(a3, x1, qsin)
                nc.vector.tensor_mul(a4, x2, qcos)
                nc.vector.tensor_sub(x1, a1, a2)
                nc.vector.tensor_add(x2, a3, a4)
            kx1 = kvv[:, :, 0:32]
            kx2 = kvv[:, :, 32:64]
            kcos = csc[:, :, 0:32]
            ksin = csc[:, :, 64:96]
            k1 = rope_tmp.tile([128, 4, 32], dt.bfloat16, tag="k1")
            k2 = rope_tmp.tile([128, 4, 32], dt.bfloat16, tag="k2")
            k3 = rope_tmp.tile([128, 4, 32], dt.bfloat16, tag="k3")
            k4 = rope_tmp.tile([128, 4, 32], dt.bfloat16, tag="k4")
            nc.vector.tensor_mul(k1[:], kx1, kcos)
            nc.vector.tensor_mul(k2[:], kx2, ksin)
            nc.vector.tensor_mul(k3[:], kx1, ksin)
            nc.vector.tensor_mul(k4[:], kx2, kcos)
            nc.vector.tensor_sub(kx1, k1[:], k2[:])
            nc.vector.tensor_add(kx2, k3[:], k4[:])
            nc.scalar.copy(kvv[:, :, 64:128], kvv[:, :, 0:64])

            qt = persist.tile([128, 4, 2, 128], dt.bfloat16, name=f"qt{jc}")
            nc.sync.dma_start_transpose(qt[:], q01[:].rearrange("p a b -> p (a b)"))
            ktT = persist.tile([128, 4, 2, 128], dt.bfloat16, name=f"ktT{jc}")
            nc.sync.dma_start_transpose(ktT[:], kvv[:].rearrange("p a b -> p (a b)"))
            qt_sb.append(qt)
            kt_sb.append(ktT)

        def emit_attention(j, hps=(0, 1), defer_last_norm=False):
            deferred = None
            for hp in hps:
                y_ps = y_ps_pool.tile(
                    [65, 2, 512], dt.float32, tag=f"y{hp}", name=f"y_ps{hp}"
                )
                last = 4 * j + 3
                pending = []  # deferred y matmuls: (i, ic, i4, off, p_t)

                def flush_y():
                    i, ic, i4, off, p_parts = pending.pop(0)
                    for u in range(2):
                        nc.tensor.matmul(
                            y_ps[:, u, off:512],
                            lhsT=kvv_sb[ic][:, i4, 128:193],
                            rhs=p_parts[u][:, off:512],
                            start=(i == 0),
                            stop=(i == last),
                        )

                for i in range(4 * j + 4):  # k tiles
                    ic, i4 = divmod(i, 4)
                    off = max(0, 128 * i - 512 * j)
                    w = 512 - off
                    p_parts = []
                    for u in range(2):  # head 2hp+u; kT copy at partitions 64u
                        s_ps = s_ps_pool.tile(
                            [128, 512], dt.float32, tag="s", name="s_ps"
                        )
                        nc.tensor.matmul(
                            s_ps[:, off:512],
                            lhsT=kt_sb[ic][ts(u, 64), i4, 0, :],
                            rhs=qt_sb[j][ts(u, 64), off // 128 : 4, hp, :],
                            start=True,
                            stop=True,
                        )
                        # route exp: ScalarE exact vs DVE Schraudolph
                        route["tot"] += w
                        use_dve = route["dve"] < EXP_DVE_FRAC * route["tot"]
                        if use_dve:
                            route["dve"] += w
                            p_u = p_pool.tile([128, 512], dt.uint16, name="p_u")
                            nc.vector.tensor_scalar(
                                p_u[:, off:512],
                                s_ps[:, off:512],
                                EXP_A,
                                EXP_B,
                                mybir.AluOpType.mult,
                                mybir.AluOpType.add,
                            )
                            p_t = p_u[:].bitcast(dt.bfloat16)
                        else:
                            p_b = p_pool.tile([128, 512], dt.bfloat16, name="p_b")
                            nc.scalar.activation(
                                p_b[:, off:512], s_ps[:, off:512], AF.Exp, scale=0.125
                            )
                            p_t = p_b[:]
                        if 128 * i >= 512 * j:  # diagonal block: causal mask
                            nc.gpsimd.affine_select(
                                p_t[:, off : off + 128],
                                p_t[:, off : off + 128],
                                pattern=[[1, 128]],
                                compare_op=mybir.AluOpType.is_ge,
                                fill=0.0,
                                base=0,
                                channel_multiplier=-1,
                            )
                        p_parts.append(p_t)
                    pending.append((i, ic, i4, off, p_parts))
                    if len(pending) > 1:
                        flush_y()
                while pending:
                    flush_y()
                # y rows 0:64 = v dims, row 64 = denominator.  The muls
                # write ynt's upper/lower partition halves DIRECTLY (DVE APs
                # carry independent partition bases) - no staging DMA.
                def emit_norm(hp=hp, y_ps=y_ps):
                    den = den_pool.tile([1, 2, 512], dt.float32)
                    nc.vector.reciprocal_approx_fast(den[:], y_ps[64:65, :, :])
                    bc = bc_pool.tile([64, 2, 512], dt.float32)
                    nc.gpsimd.partition_broadcast(bc[:], den[:], channels=64)
                    for u in range(2):
                        yst = yst_pool.tile([64, 512], dt.bfloat16)
                        nc.vector.tensor_mul(yst[:, :], y_ps[0:64, u, :], bc[:, u, :])
                        nc.scalar.dma_start(ynt[hp][j][ts(u, 64), :], yst[:])
                if defer_last_norm and hp == hps[-1]:
                    deferred = emit_norm
                else:
                    emit_norm()
            return deferred

        def emit_proj(j):
            for t4 in range(4):
                tt = 4 * j + t4
                po = po_pool.tile([128, 1024], dt.bfloat16)
                for nn2 in range(2):
                    ps = s_ps_pool.tile([128, 512], dt.float32, tag="s", name="pr_ps")
                    for dtile in range(2):
                        nc.tensor.matmul(
                            ps[:],
                            lhsT=ynt[dtile][j][:, ts(t4, 128)],
                            rhs=wo_sb[:, dtile, ts(nn2, 512)],
                            start=(dtile == 0),
                            stop=(dtile == 1),
                        )
                    if nn2 == 0:
                        nc.scalar.copy(po[:, ts(nn2, 512)], ps[:])
                    else:
                        nc.vector.tensor_copy(po[:, ts(nn2, 512)], ps[:])
                nc.scalar.dma_start(out_d.ap()[ts(tt, 128), :], po[:])

        # ---- interleaved emission: each engine-queue boundary is covered by
        # independent PE work (qkv of a later chunk, proj of an earlier one);
        # attention(j) is emitted before phase1(j+1) so its score matmuls
        # never wait on later chunks' transposes (per-queue coalesced deps) --
        emit_phase1(0)
        emit_phase1(1)
        emit_attention(0)
        n1 = emit_attention(1, defer_last_norm=True)
        emit_phase1(2)
        emit_proj(0)
        n1()
        emit_phase1(3)
        n2 = emit_attention(2, defer_last_norm=True)
        emit_proj(1)
        n2()
        n30 = emit_attention(3, hps=(0,), defer_last_norm=True)
        emit_proj(2)
        n30()
        emit_attention(3, hps=(1,))
        emit_proj(3)

    nc.finalize()
    return nc


def _host_inputs(x, W_qkv, W_proj):
    """Per-core input maps (host-side sharding + partition-major layout)."""
    perm = np.concatenate([np.arange(0, HD, 2), np.arange(1, HD, 2)])  # even|odd
    inv = 1.0 / THETA ** (np.arange(0, HD, 2, dtype=np.float64) / HD)  # [32]
    ang = np.arange(T, dtype=np.float64)[:, None] * inv[None, :]       # [T, 32]
    cos2 = np.tile(np.cos(ang), (1, 2))
    sin2 = np.tile(np.sin(ang), (1, 2))
    cs = np.concatenate([cos2, sin2], axis=1).astype(BF16)             # [T, 128]
    cs_pm = np.ascontiguousarray(
        cs.reshape(TT, 128, 128).transpose(1, 0, 2).reshape(128, TT * 128)
    )

    def part_major(a, p=128):
        R, cols = a.shape
        n = R // p
        return np.ascontiguousarray(
            a.reshape(n, p, cols).transpose(1, 0, 2).reshape(p, n * cols)
        )

    in_maps = []
    for core in range(8):
        b, hg = divmod(core, 4)
        xT = x[b].T.astype(BF16)                                       # [C, T]
        xt_pm = np.ascontiguousarray(
            xT.reshape(8, 128, NCHUNK, 512)
            .transpose(1, 2, 0, 3)
            .reshape(128, NCHUNK * 8 * 512)
        )
        cols = []
        for h in range(hg * NQ, hg * NQ + NQ):
            cols.append(W_qkv[:, h * HD : (h + 1) * HD][:, perm])
        kblk = W_qkv[:, NH * HD + hg * HD : NH * HD + (hg + 1) * HD][:, perm]
        vblk = W_qkv[:, (NH + NKV) * HD + hg * HD : (NH + NKV) * HD + (hg + 1) * HD]
        w384 = np.concatenate(cols + [kblk, vblk], axis=1).astype(BF16)
        wo = W_proj[hg * NQ * HD : (hg + 1) * NQ * HD, :].astype(BF16)
        in_maps.append(
            {
                "xT": xt_pm,
                "w384": part_major(w384),
                "wo": part_major(wo),
                "cs": cs_pm,
            }
        )
    return in_maps


def _run(in_maps):
    from concourse.bass_utils import run_bass_kernel_spmd

    if "nc" not in _CACHE:
        _CACHE["nc"] = _build()
    return run_bass_kernel_spmd(_CACHE["nc"], in_maps, core_ids=list(range(8)))


def kernel(x, W_qkv, W_proj):
    x = np.asarray(x, dtype=np.float32)
    W_qkv = np.asarray(W_qkv, dtype=np.float32)
    W_proj = np.asarray(W_proj, dtype=np.float32)
    res = _run(_host_inputs(x, W_qkv, W_proj))
    out = np.zeros((B, T, C), dtype=np.float32)
    for core in range(8):
        b = core // 4
        out[b] += res.results[core]["out"].astype(np.float32)
    return out
